# revision 23
# baseline (speedup 1.0000x reference)
"""Trainium2 Bass kernel for quantized CustomConv2d.

Computes, for full inputs
    x_q  (32, 128, 56, 56) f32 (int8-valued)
    w_q  (256, 128, 3, 3)  f32 (int8-valued)
    bias (256,)            f32
the reference:
    acc = conv2d(x_q, w_q, stride 1, pad 1) + bias
    out = clip(round(acc / 128), -128, 127); out = max(out, 0)   # fused ReLU, zp=0
returning (32, 256, 56, 56) f32.

Strategy: data-parallel over batch (4 images per core on 8 cores).
Per core the conv is 9 shifted matmuls with C=128 on SBUF partitions,
accumulated in PSUM over (r, s); out-channels split in 2 chunks of 128
(PSUM partition dim). Inputs cast to bf16 on host (exact for int8-valued
data); accumulation is fp32 in PSUM, so the integer conv result is exact.
Requantization fuses into: ACT relu(acc/128 + b/128) -> DVE round-to-
nearest-even via the +1.5*2^23 magic trick with upper clamp -> int8 DMA out.
Host casts int8 back to f32. Result is bit-exact vs the fp32 reference.
"""

import sys

sys.path.insert(0, "/opt/trn_rl_repo")

import ml_dtypes
import numpy as np

import concourse.bass as bass
import concourse.mybir as mybir
import concourse.tile as tile
from concourse import bacc
from concourse import bass_utils

# Problem geometry (hardcoded per contract)
N, C, H, W = 32, 128, 56, 56
K = 256
NCORES = 8
NPC = N // NCORES          # images per core
HP, WP = H + 2, W + 2      # padded input plane
KC = K // 128              # out-channel chunks of 128
TR = 8                     # output rows per psum tile
RT = H // TR               # row tiles per image
FREE = TR * W              # 448 fp32 -> fits one PSUM bank
MAGIC = 12582912.0         # 1.5 * 2**23: RNE integer rounding for |v| < 2**22

_CACHE = {}


def _build_program():
    nc = bacc.Bacc("TRN2", target_bir_lowering=False, debug=False)
    x_in = nc.dram_tensor("x", [C, NPC, HP, WP], mybir.dt.bfloat16, kind="ExternalInput")
    w_in = nc.dram_tensor("w", [C, 9 * KC, 128], mybir.dt.bfloat16, kind="ExternalInput")
    b_in = nc.dram_tensor("b", [128, KC], mybir.dt.float32, kind="ExternalInput")
    y_out = nc.dram_tensor("y", [NPC, KC, 128, RT, FREE], mybir.dt.int8, kind="ExternalOutput")

    # x DMA row-chunks: [start_row, end_row) covering row tiles [rt_lo, rt_hi)
    # (tile rt reads padded rows rt*TR .. rt*TR+TR+1 inclusive)
    CHUNKS = [(0, 10, 0, 1), (8, 26, 1, 3), (24, 42, 3, 5), (40, 58, 5, 7)]
    N_WARMUP = 60  # dummy matmuls to hold/warm the PE during the input-DMA head

    with tile.TileContext(nc) as tc:
        with (
            tc.tile_pool(name="xin", bufs=1) as xpool,
            tc.tile_pool(name="wb", bufs=1) as wbpool,
            tc.tile_pool(name="ps", bufs=7, space="PSUM") as pspool,
            tc.tile_pool(name="wup", bufs=1, space="PSUM") as wuppool,
            tc.tile_pool(name="post", bufs=4) as postpool,
            tc.tile_pool(name="oi8", bufs=4) as opool,
        ):
            # PE warmup: matmuls on scratch SBUF into a scratch PSUM bank.
            # They have no DMA dependencies, so the PE starts immediately and
            # the HAM clock-gate reaches 2.4 GHz while inputs stream in.
            scr = wbpool.tile([128, 128], mybir.dt.bfloat16, tag="scr")
            nc.gpsimd.memset(scr[:], 0.0)
            wup = wuppool.tile([64, 64], mybir.dt.float32, tag="wup")
            for _ in range(N_WARMUP):
                nc.tensor.matmul(wup[:], scr[:, 0:64], scr[:, 0:64], start=True, stop=True)

            # Input DMAs spread across engine queues so transfers run in
            # parallel; w is split by kc half into separate tiles so the
            # first tiles' matmuls only wait on their own half (tile-level
            # dependency tracking). Layout is [kc, r, s] along the middle axis.
            # wt0 (needed by the very first tiles) striped across sync+scalar
            # queues; wt1 on sync behind wt0's half.
            # The critical set for the first tiles is w0 (295KB) + x00 (148KB):
            # stripe both across sync/scalar/gpsimd so the head is ~queue-parallel.
            xts = {}
            x00 = xpool.tile([C, 10, WP], mybir.dt.bfloat16, tag="x0c0")
            nc.sync.dma_start(x00[:, 0:4, :], x_in[:, 0, 0:4])
            nc.scalar.dma_start(x00[:, 4:7, :], x_in[:, 0, 4:7])
            nc.gpsimd.dma_start(x00[:, 7:10, :], x_in[:, 0, 7:10])
            xts[(0, 0)] = x00

            wts = []
            wt0 = wbpool.tile([C, 9, 128], mybir.dt.bfloat16, tag="wt0")
            nc.sync.dma_start(wt0[:, 0:4, :], w_in[:, 0:4, :])
            nc.scalar.dma_start(wt0[:, 4:7, :], w_in[:, 4:7, :])
            nc.gpsimd.dma_start(wt0[:, 7:9, :], w_in[:, 7:9, :])
            wts.append(wt0)
            wt1 = wbpool.tile([C, 9, 128], mybir.dt.bfloat16, tag="wt1")
            nc.sync.dma_start(wt1[:], w_in[:, 9:18, :])
            wts.append(wt1)
            bt = wbpool.tile([128, KC], mybir.dt.float32, tag="bt")
            nc.gpsimd.dma_start(bt[:], b_in[:])

            for n in range(NPC):
                for ci, (r0, r1, _, _) in enumerate(CHUNKS):
                    if (n, ci) in xts:
                        continue
                    xt = xpool.tile([C, r1 - r0, WP], mybir.dt.bfloat16, tag=f"x{n}c{ci}")
                    eng = nc.gpsimd if n < 2 else nc.scalar
                    eng.dma_start(xt[:], x_in[:, n, r0:r1])
                    xts[(n, ci)] = xt

            for n in range(NPC):
                for ci, (r0, r1, rt_lo, rt_hi) in enumerate(CHUNKS):
                    for rt in range(rt_lo, rt_hi):
                        for kc in range(KC):
                            ps = pspool.tile([128, FREE], mybir.dt.float32, tag="ps")
                            for r in range(3):
                                for s in range(3):
                                    row = rt * TR + r - r0
                                    nc.tensor.matmul(
                                        ps[:, :],
                                        wts[kc][:, r * 3 + s, :],
                                        xts[(n, ci)][:, row : row + TR, s : s + W],
                                        start=(r == 0 and s == 0),
                                        stop=(r == 2 and s == 2),
                                    )
                            act = postpool.tile([128, FREE], mybir.dt.float32, tag="act")
                            nc.scalar.activation(
                                act[:],
                                ps[:],
                                mybir.ActivationFunctionType.Relu,
                                bias=bt[:, kc : kc + 1],
                                scale=1.0 / 128.0,
                            )
                            rnd = postpool.tile([128, FREE], mybir.dt.float32, tag="rnd")
                            nc.vector.tensor_scalar(
                                rnd[:],
                                act[:],
                                MAGIC,
                                MAGIC + 127.0,
                                mybir.AluOpType.add,
                                mybir.AluOpType.min,
                            )
                            oi8 = opool.tile([128, FREE], mybir.dt.int8, tag="oi8")
                            nc.vector.tensor_scalar(
                                oi8[:], rnd[:], -MAGIC, None, mybir.AluOpType.add
                            )
                            nc.sync.dma_start(y_out[n, kc, :, rt, :], oi8[:])
    nc.compile()
    return nc


def _prep_inputs(x_q, w_q, bias):
    x = np.ascontiguousarray(x_q, dtype=np.float32)
    # pad H, W by 1 -> (N, C, 58, 58), cast bf16 (exact: values are ints <= 128)
    xp = np.zeros((N, C, HP, WP), dtype=ml_dtypes.bfloat16)
    xp[:, :, 1 : H + 1, 1 : W + 1] = x
    w = np.ascontiguousarray(w_q, dtype=np.float32).reshape(KC, 128, C, 3, 3)
    # wt[c, kc*9 + r*3+s, kl] = w_q[kc*128+kl, c, r, s]
    wt = np.ascontiguousarray(w.transpose(2, 0, 3, 4, 1)).reshape(C, 9 * KC, 128)
    wt = wt.astype(ml_dtypes.bfloat16)
    bt = np.ascontiguousarray(
        (np.asarray(bias, dtype=np.float32).reshape(KC, 128) / 128.0).T
    )
    in_maps = []
    for core in range(NCORES):
        xc = xp[core * NPC : (core + 1) * NPC]          # (NPC, C, HP, WP)
        xc = np.ascontiguousarray(xc.transpose(1, 0, 2, 3))  # (C, NPC, HP, WP)
        in_maps.append({"x": xc, "w": wt, "b": bt})
    return in_maps


def _postprocess(results):
    outs = []
    for core in range(NCORES):
        y = results[core]["y"]  # (NPC, KC, 128, RT, FREE) int8
        outs.append(y.reshape(NPC, K, H, W).astype(np.float32))
    return np.concatenate(outs, axis=0)


def run(x_q, w_q, bias, **run_kwargs):
    """Build (cached), run on 8 cores, return full output + raw results."""
    if "nc" not in _CACHE:
        _CACHE["nc"] = _build_program()
    nc = _CACHE["nc"]
    in_maps = _prep_inputs(x_q, w_q, bias)
    res = bass_utils.run_bass_kernel_spmd(
        nc, in_maps, list(range(NCORES)), **run_kwargs
    )
    return _postprocess(res.results), res


def kernel(x_q, w_q, bias):
    out, _ = run(x_q, w_q, bias)
    return out


# revision 25
# speedup vs baseline: 1.1996x; 1.1996x over previous
"""Trainium2 Bass kernel for quantized CustomConv2d.

Computes, for full inputs
    x_q  (32, 128, 56, 56) f32 (int8-valued)
    w_q  (256, 128, 3, 3)  f32 (int8-valued)
    bias (256,)            f32
the reference:
    acc = conv2d(x_q, w_q, stride 1, pad 1) + bias
    out = clip(round(acc / 128), -128, 127); out = max(out, 0)   # fused ReLU, zp=0
returning (32, 256, 56, 56) f32.

Strategy: data-parallel over batch (4 images per core on 8 cores).
Per core the conv is 9 shifted matmuls with C=128 on SBUF partitions,
accumulated in PSUM over (r, s); out-channels split in 2 chunks of 128
(PSUM partition dim). Inputs cast to bf16 on host (exact for int8-valued
data); accumulation is fp32 in PSUM, so the integer conv result is exact.
Requantization fuses into: ACT relu(acc/128 + b/128) -> DVE round-to-
nearest-even via the +1.5*2^23 magic trick with upper clamp -> int8 DMA out.
Host casts int8 back to f32. Result is bit-exact vs the fp32 reference.
"""

import sys

sys.path.insert(0, "/opt/trn_rl_repo")

import ml_dtypes
import numpy as np

import concourse.bass as bass
import concourse.mybir as mybir
import concourse.tile as tile
from concourse import bacc
from concourse import bass_utils

# Problem geometry (hardcoded per contract)
N, C, H, W = 32, 128, 56, 56
K = 256
NCORES = 8
NPC = N // NCORES          # images per core
HP, WP = H + 2, W + 2      # padded input plane
KC = K // 128              # out-channel chunks of 128
TR = 8                     # output rows per psum tile
RT = H // TR               # row tiles per image
FREE = TR * W              # 448 fp32 -> fits one PSUM bank
MAGIC = 12582912.0         # 1.5 * 2**23: RNE integer rounding for |v| < 2**22

_CACHE = {}


def _build_program():
    nc = bacc.Bacc("TRN2", target_bir_lowering=False, debug=False)
    x_in = nc.dram_tensor("x", [C, NPC, HP, WP], mybir.dt.bfloat16, kind="ExternalInput")
    w_in = nc.dram_tensor("w", [C, 9 * KC, 128], mybir.dt.bfloat16, kind="ExternalInput")
    b_in = nc.dram_tensor("b", [128, KC], mybir.dt.float32, kind="ExternalInput")
    y_out = nc.dram_tensor("y", [NPC, KC, 128, RT, FREE], mybir.dt.int8, kind="ExternalOutput")

    # x DMA row-chunks: [start_row, end_row) covering row tiles [rt_lo, rt_hi)
    # (tile rt reads padded rows rt*TR .. rt*TR+TR+1 inclusive)
    CHUNKS = [(0, 10, 0, 1), (8, 26, 1, 3), (24, 42, 3, 5), (40, 58, 5, 7)]
    N_WARMUP = 60  # dummy matmuls to hold/warm the PE during the input-DMA head

    with tile.TileContext(nc) as tc:
        with (
            tc.tile_pool(name="xin", bufs=1) as xpool,
            tc.tile_pool(name="wb", bufs=1) as wbpool,
            tc.tile_pool(name="ps", bufs=7, space="PSUM") as pspool,
            tc.tile_pool(name="wup", bufs=1, space="PSUM") as wuppool,
            tc.tile_pool(name="post", bufs=4) as postpool,
            tc.tile_pool(name="oi8", bufs=12) as opool,
        ):
            # PE warmup: matmuls on scratch SBUF into a scratch PSUM bank.
            # They have no DMA dependencies, so the PE starts immediately and
            # the HAM clock-gate reaches 2.4 GHz while inputs stream in.
            scr = wbpool.tile([128, 128], mybir.dt.bfloat16, tag="scr")
            nc.gpsimd.memset(scr[:], 0.0)
            wup = wuppool.tile([64, 64], mybir.dt.float32, tag="wup")
            for _ in range(N_WARMUP):
                nc.tensor.matmul(wup[:], scr[:, 0:64], scr[:, 0:64], start=True, stop=True)

            # Input DMAs spread across engine queues so transfers run in
            # parallel; w is split by kc half into separate tiles so the
            # first tiles' matmuls only wait on their own half (tile-level
            # dependency tracking). Layout is [kc, r, s] along the middle axis.
            # wt0 (needed by the very first tiles) striped across sync+scalar
            # queues; wt1 on sync behind wt0's half.
            # kc-outer compute order keeps w1 (295KB) out of the critical
            # first-12us DMA window. Queue schedule (sync/scalar ~65GB/s
            # HWDGE, gpsimd ~45GB/s SWDGE), each queue ordered by need-time:
            #   sync:   x00a w0a x01a x02a x03a w1a | all outputs
            #   scalar: x00b w0b x01b x02b x03b w1b | x2* x3*
            #   gpsimd: x00c w0c b                  | x1*
            xts = {}
            x00 = xpool.tile([C, 10, WP], mybir.dt.bfloat16, tag="x0c0")
            nc.sync.dma_start(x00[:, 0:4, :], x_in[:, 0, 0:4])
            nc.scalar.dma_start(x00[:, 4:7, :], x_in[:, 0, 4:7])
            nc.gpsimd.dma_start(x00[:, 7:10, :], x_in[:, 0, 7:10])
            xts[(0, 0)] = x00

            wts = []
            wt0 = wbpool.tile([C, 9, 128], mybir.dt.bfloat16, tag="wt0")
            nc.sync.dma_start(wt0[:, 0:4, :], w_in[:, 0:4, :])
            nc.scalar.dma_start(wt0[:, 4:7, :], w_in[:, 4:7, :])
            nc.gpsimd.dma_start(wt0[:, 7:9, :], w_in[:, 7:9, :])
            wts.append(wt0)
            bt = wbpool.tile([128, KC], mybir.dt.float32, tag="bt")
            nc.gpsimd.dma_start(bt[:], b_in[:])

            # image-0 chunks 1..3 striped across the two fast queues
            for ci, (r0, r1, _, _) in enumerate(CHUNKS):
                if ci == 0:
                    continue
                xt = xpool.tile([C, r1 - r0, WP], mybir.dt.bfloat16, tag=f"x0c{ci}")
                mid = (r0 + r1) // 2
                nc.sync.dma_start(xt[:, 0 : mid - r0, :], x_in[:, 0, r0:mid])
                nc.scalar.dma_start(xt[:, mid - r0 :, :], x_in[:, 0, mid:r1])
                xts[(0, ci)] = xt

            wt1 = wbpool.tile([C, 9, 128], mybir.dt.bfloat16, tag="wt1")
            nc.sync.dma_start(wt1[:, 0:5, :], w_in[:, 9:14, :])
            nc.scalar.dma_start(wt1[:, 5:9, :], w_in[:, 14:18, :])
            wts.append(wt1)

            for n in range(1, NPC):
                for ci, (r0, r1, _, _) in enumerate(CHUNKS):
                    xt = xpool.tile([C, r1 - r0, WP], mybir.dt.bfloat16, tag=f"x{n}c{ci}")
                    eng = nc.gpsimd if n == 1 else nc.scalar
                    eng.dma_start(xt[:], x_in[:, n, r0:r1])
                    xts[(n, ci)] = xt

            rt2chunk = {rt: ci for ci, (_, _, lo, hi) in enumerate(CHUNKS) for rt in range(lo, hi)}
            tiles = [(n, kc, rt) for n in range(NPC) for kc in range(KC) for rt in range(RT)]
            for ti, (n, kc, rt) in enumerate(tiles):
                ci = rt2chunk[rt]
                r0 = CHUNKS[ci][0]
                last = ti == len(tiles) - 1
                # split the final tile so its serial requant tail is halved
                halves = ((0, FREE // 2), (FREE // 2, FREE)) if last else ((0, FREE),)
                for hv, (lo, hi) in enumerate(halves):
                    width = hi - lo
                    ps = pspool.tile([128, FREE], mybir.dt.float32, tag="ps")
                    for r in range(3):
                        for s in range(3):
                            row = rt * TR + r - r0 + lo // W
                            nc.tensor.matmul(
                                ps[:, 0:width],
                                wts[kc][:, r * 3 + s, :],
                                xts[(n, ci)][:, row : row + width // W, s : s + W],
                                start=(r == 0 and s == 0),
                                stop=(r == 2 and s == 2),
                            )
                    act = postpool.tile([128, FREE], mybir.dt.float32, tag="act")
                    nc.scalar.activation(
                        act[:, 0:width],
                        ps[:, 0:width],
                        mybir.ActivationFunctionType.Relu,
                        bias=bt[:, kc : kc + 1],
                        scale=1.0 / 128.0,
                    )
                    rnd = postpool.tile([128, FREE], mybir.dt.float32, tag="rnd")
                    nc.vector.tensor_scalar(
                        rnd[:, 0:width],
                        act[:, 0:width],
                        MAGIC,
                        MAGIC + 127.0,
                        mybir.AluOpType.add,
                        mybir.AluOpType.min,
                    )
                    oi8 = opool.tile([128, FREE], mybir.dt.int8, tag=f"oi8{hv}")
                    nc.vector.tensor_scalar(
                        oi8[:, 0:width], rnd[:, 0:width], -MAGIC, None, mybir.AluOpType.add
                    )
                    nc.sync.dma_start(y_out[n, kc, :, rt, lo:hi], oi8[:, 0:width])
    nc.compile()
    return nc


def _prep_inputs(x_q, w_q, bias):
    x = np.ascontiguousarray(x_q, dtype=np.float32)
    # pad H, W by 1 -> (N, C, 58, 58), cast bf16 (exact: values are ints <= 128)
    xp = np.zeros((N, C, HP, WP), dtype=ml_dtypes.bfloat16)
    xp[:, :, 1 : H + 1, 1 : W + 1] = x
    w = np.ascontiguousarray(w_q, dtype=np.float32).reshape(KC, 128, C, 3, 3)
    # wt[c, kc*9 + r*3+s, kl] = w_q[kc*128+kl, c, r, s]
    wt = np.ascontiguousarray(w.transpose(2, 0, 3, 4, 1)).reshape(C, 9 * KC, 128)
    wt = wt.astype(ml_dtypes.bfloat16)
    bt = np.ascontiguousarray(
        (np.asarray(bias, dtype=np.float32).reshape(KC, 128) / 128.0).T
    )
    in_maps = []
    for core in range(NCORES):
        xc = xp[core * NPC : (core + 1) * NPC]          # (NPC, C, HP, WP)
        xc = np.ascontiguousarray(xc.transpose(1, 0, 2, 3))  # (C, NPC, HP, WP)
        in_maps.append({"x": xc, "w": wt, "b": bt})
    return in_maps


def _postprocess(results):
    outs = []
    for core in range(NCORES):
        y = results[core]["y"]  # (NPC, KC, 128, RT, FREE) int8
        outs.append(y.reshape(NPC, K, H, W).astype(np.float32))
    return np.concatenate(outs, axis=0)


def run(x_q, w_q, bias, **run_kwargs):
    """Build (cached), run on 8 cores, return full output + raw results."""
    if "nc" not in _CACHE:
        _CACHE["nc"] = _build_program()
    nc = _CACHE["nc"]
    in_maps = _prep_inputs(x_q, w_q, bias)
    res = bass_utils.run_bass_kernel_spmd(
        nc, in_maps, list(range(NCORES)), **run_kwargs
    )
    return _postprocess(res.results), res


def kernel(x_q, w_q, bias):
    out, _ = run(x_q, w_q, bias)
    return out


# revision 26
# speedup vs baseline: 1.2045x; 1.0041x over previous
"""Trainium2 Bass kernel for quantized CustomConv2d.

Computes, for full inputs
    x_q  (32, 128, 56, 56) f32 (int8-valued)
    w_q  (256, 128, 3, 3)  f32 (int8-valued)
    bias (256,)            f32
the reference:
    acc = conv2d(x_q, w_q, stride 1, pad 1) + bias
    out = clip(round(acc / 128), -128, 127); out = max(out, 0)   # fused ReLU, zp=0
returning (32, 256, 56, 56) f32.

Strategy: data-parallel over batch (4 images per core on 8 cores).
Per core the conv is 9 shifted matmuls with C=128 on SBUF partitions,
accumulated in PSUM over (r, s); out-channels split in 2 chunks of 128
(PSUM partition dim). Inputs cast to bf16 on host (exact for int8-valued
data); accumulation is fp32 in PSUM, so the integer conv result is exact.
Requantization fuses into: ACT relu(acc/128 + b/128) -> DVE round-to-
nearest-even via the +1.5*2^23 magic trick with upper clamp -> int8 DMA out.
Host casts int8 back to f32. Result is bit-exact vs the fp32 reference.
"""

import sys

sys.path.insert(0, "/opt/trn_rl_repo")

import ml_dtypes
import numpy as np

import concourse.bass as bass
import concourse.mybir as mybir
import concourse.tile as tile
from concourse import bacc
from concourse import bass_utils

# Problem geometry (hardcoded per contract)
N, C, H, W = 32, 128, 56, 56
K = 256
NCORES = 8
NPC = N // NCORES          # images per core
HP, WP = H + 2, W + 2      # padded input plane
KC = K // 128              # out-channel chunks of 128
TR = 8                     # output rows per psum tile
RT = H // TR               # row tiles per image
FREE = TR * W              # 448 fp32 -> fits one PSUM bank
MAGIC = 12582912.0         # 1.5 * 2**23: RNE integer rounding for |v| < 2**22

_CACHE = {}


def _build_program():
    nc = bacc.Bacc("TRN2", target_bir_lowering=False, debug=False)
    x_in = nc.dram_tensor("x", [C, NPC, HP, WP], mybir.dt.bfloat16, kind="ExternalInput")
    w_in = nc.dram_tensor("w", [C, 9 * KC, 128], mybir.dt.bfloat16, kind="ExternalInput")
    b_in = nc.dram_tensor("b", [128, KC], mybir.dt.float32, kind="ExternalInput")
    y_out = nc.dram_tensor("y", [NPC, KC, 128, RT, FREE], mybir.dt.int8, kind="ExternalOutput")

    # x DMA row-chunks: [start_row, end_row) covering row tiles [rt_lo, rt_hi)
    # (tile rt reads padded rows rt*TR .. rt*TR+TR+1 inclusive)
    CHUNKS = [(0, 10, 0, 1), (8, 26, 1, 3), (24, 42, 3, 5), (40, 58, 5, 7)]
    N_WARMUP = 70  # dummy matmuls to hold/warm the PE during the input-DMA head

    with tile.TileContext(nc) as tc:
        with (
            tc.tile_pool(name="xin", bufs=1) as xpool,
            tc.tile_pool(name="wb", bufs=1) as wbpool,
            tc.tile_pool(name="ps", bufs=7, space="PSUM") as pspool,
            tc.tile_pool(name="wup", bufs=1, space="PSUM") as wuppool,
            tc.tile_pool(name="post", bufs=4) as postpool,
            tc.tile_pool(name="oi8", bufs=12) as opool,
        ):
            # PE warmup: matmuls on scratch SBUF into a scratch PSUM bank.
            # They have no DMA dependencies, so the PE starts immediately and
            # the HAM clock-gate reaches 2.4 GHz while inputs stream in.
            scr = wbpool.tile([128, 128], mybir.dt.bfloat16, tag="scr")
            nc.gpsimd.memset(scr[:], 0.0)
            wup = wuppool.tile([64, 64], mybir.dt.float32, tag="wup")
            for _ in range(N_WARMUP):
                nc.tensor.matmul(wup[:], scr[:, 0:64], scr[:, 0:64], start=True, stop=True)

            # Input DMAs spread across engine queues so transfers run in
            # parallel; w is split by kc half into separate tiles so the
            # first tiles' matmuls only wait on their own half (tile-level
            # dependency tracking). Layout is [kc, r, s] along the middle axis.
            # wt0 (needed by the very first tiles) striped across sync+scalar
            # queues; wt1 on sync behind wt0's half.
            # kc-outer compute order keeps w1 (295KB) out of the critical
            # first-12us DMA window. Queue schedule (sync/scalar ~65GB/s
            # HWDGE, gpsimd ~45GB/s SWDGE), each queue ordered by need-time:
            #   sync:   x00a w0a x01a x02a x03a w1a | all outputs
            #   scalar: x00b w0b x01b x02b x03b w1b | x2* x3*
            #   gpsimd: x00c w0c b                  | x1*
            xts = {}
            x00 = xpool.tile([C, 10, WP], mybir.dt.bfloat16, tag="x0c0")
            nc.sync.dma_start(x00[:, 0:4, :], x_in[:, 0, 0:4])
            nc.scalar.dma_start(x00[:, 4:7, :], x_in[:, 0, 4:7])
            nc.gpsimd.dma_start(x00[:, 7:10, :], x_in[:, 0, 7:10])
            xts[(0, 0)] = x00

            wts = []
            wt0 = wbpool.tile([C, 9, 128], mybir.dt.bfloat16, tag="wt0")
            nc.sync.dma_start(wt0[:, 0:4, :], w_in[:, 0:4, :])
            nc.scalar.dma_start(wt0[:, 4:7, :], w_in[:, 4:7, :])
            nc.gpsimd.dma_start(wt0[:, 7:9, :], w_in[:, 7:9, :])
            wts.append(wt0)
            bt = wbpool.tile([128, KC], mybir.dt.float32, tag="bt")
            nc.gpsimd.dma_start(bt[:], b_in[:])

            # image-0 chunks 1..3 striped across the two fast queues
            for ci, (r0, r1, _, _) in enumerate(CHUNKS):
                if ci == 0:
                    continue
                xt = xpool.tile([C, r1 - r0, WP], mybir.dt.bfloat16, tag=f"x0c{ci}")
                mid = (r0 + r1) // 2
                nc.sync.dma_start(xt[:, 0 : mid - r0, :], x_in[:, 0, r0:mid])
                nc.scalar.dma_start(xt[:, mid - r0 :, :], x_in[:, 0, mid:r1])
                xts[(0, ci)] = xt

            wt1 = wbpool.tile([C, 9, 128], mybir.dt.bfloat16, tag="wt1")
            nc.sync.dma_start(wt1[:, 0:5, :], w_in[:, 9:14, :])
            nc.scalar.dma_start(wt1[:, 5:9, :], w_in[:, 14:18, :])
            wts.append(wt1)

            for n in range(1, NPC):
                for ci, (r0, r1, _, _) in enumerate(CHUNKS):
                    xt = xpool.tile([C, r1 - r0, WP], mybir.dt.bfloat16, tag=f"x{n}c{ci}")
                    eng = nc.gpsimd if n == 1 else nc.scalar
                    eng.dma_start(xt[:], x_in[:, n, r0:r1])
                    xts[(n, ci)] = xt

            rt2chunk = {rt: ci for ci, (_, _, lo, hi) in enumerate(CHUNKS) for rt in range(lo, hi)}
            tiles = [(n, kc, rt) for n in range(NPC) for kc in range(KC) for rt in range(RT)]
            for ti, (n, kc, rt) in enumerate(tiles):
                ci = rt2chunk[rt]
                r0 = CHUNKS[ci][0]
                last = ti == len(tiles) - 1
                # split the final tile so its serial requant tail is halved
                halves = ((0, FREE // 2), (FREE // 2, FREE)) if last else ((0, FREE),)
                for hv, (lo, hi) in enumerate(halves):
                    width = hi - lo
                    ps = pspool.tile([128, FREE], mybir.dt.float32, tag="ps")
                    for r in range(3):
                        for s in range(3):
                            row = rt * TR + r - r0 + lo // W
                            nc.tensor.matmul(
                                ps[:, 0:width],
                                wts[kc][:, r * 3 + s, :],
                                xts[(n, ci)][:, row : row + width // W, s : s + W],
                                start=(r == 0 and s == 0),
                                stop=(r == 2 and s == 2),
                            )
                    act = postpool.tile([128, FREE], mybir.dt.float32, tag="act")
                    nc.scalar.activation(
                        act[:, 0:width],
                        ps[:, 0:width],
                        mybir.ActivationFunctionType.Relu,
                        bias=bt[:, kc : kc + 1],
                        scale=1.0 / 128.0,
                    )
                    rnd = postpool.tile([128, FREE], mybir.dt.float32, tag="rnd")
                    nc.vector.tensor_scalar(
                        rnd[:, 0:width],
                        act[:, 0:width],
                        MAGIC,
                        MAGIC + 127.0,
                        mybir.AluOpType.add,
                        mybir.AluOpType.min,
                    )
                    oi8 = opool.tile([128, FREE], mybir.dt.int8, tag=f"oi8{hv}")
                    nc.vector.tensor_scalar(
                        oi8[:, 0:width], rnd[:, 0:width], -MAGIC, None, mybir.AluOpType.add
                    )
                    nc.sync.dma_start(y_out[n, kc, :, rt, lo:hi], oi8[:, 0:width])
    nc.compile()
    return nc


def _prep_inputs(x_q, w_q, bias):
    x = np.ascontiguousarray(x_q, dtype=np.float32)
    # pad H, W by 1 -> (N, C, 58, 58), cast bf16 (exact: values are ints <= 128)
    xp = np.zeros((N, C, HP, WP), dtype=ml_dtypes.bfloat16)
    xp[:, :, 1 : H + 1, 1 : W + 1] = x
    w = np.ascontiguousarray(w_q, dtype=np.float32).reshape(KC, 128, C, 3, 3)
    # wt[c, kc*9 + r*3+s, kl] = w_q[kc*128+kl, c, r, s]
    wt = np.ascontiguousarray(w.transpose(2, 0, 3, 4, 1)).reshape(C, 9 * KC, 128)
    wt = wt.astype(ml_dtypes.bfloat16)
    bt = np.ascontiguousarray(
        (np.asarray(bias, dtype=np.float32).reshape(KC, 128) / 128.0).T
    )
    in_maps = []
    for core in range(NCORES):
        xc = xp[core * NPC : (core + 1) * NPC]          # (NPC, C, HP, WP)
        xc = np.ascontiguousarray(xc.transpose(1, 0, 2, 3))  # (C, NPC, HP, WP)
        in_maps.append({"x": xc, "w": wt, "b": bt})
    return in_maps


def _postprocess(results):
    outs = []
    for core in range(NCORES):
        y = results[core]["y"]  # (NPC, KC, 128, RT, FREE) int8
        outs.append(y.reshape(NPC, K, H, W).astype(np.float32))
    return np.concatenate(outs, axis=0)


def run(x_q, w_q, bias, **run_kwargs):
    """Build (cached), run on 8 cores, return full output + raw results."""
    if "nc" not in _CACHE:
        _CACHE["nc"] = _build_program()
    nc = _CACHE["nc"]
    in_maps = _prep_inputs(x_q, w_q, bias)
    res = bass_utils.run_bass_kernel_spmd(
        nc, in_maps, list(range(NCORES)), **run_kwargs
    )
    return _postprocess(res.results), res


def kernel(x_q, w_q, bias):
    out, _ = run(x_q, w_q, bias)
    return out
